# revision 49
# baseline (speedup 1.0000x reference)
"""Trainium2 Bass kernel for a pre-LN transformer block (attention + FFN).

Full inputs in, full outputs out. Data-parallel over the batch dim across
8 NeuronCores (8 batches/core), weights replicated, no collectives.

Per-core dataflow (2048 tokens = 16 tiles of 128):
  LN1 (token-major, bn_stats) -> h bf16 -> PE-transpose -> h^T [c, t]
  Q^T,K^T = Wqk^T @ h^T (feature-major [d, t]); V = h^T-stationary @ Wv
  (token-major [s, d], ones column appended for softmax denominators)
  wei^T[s,t] = K^T-slices stationary @ Q^T; E = exp(wei * C^-0.5) via ACT
  (logits are O(0.3) so no max-subtraction needed); causal mask = one
  upper-triangular multiply per diagonal block
  attn[t, 64+1] = E-stationary @ [V | 1]  (numerator + denominator in one
  accumulation, token-major) -> reciprocal + scale -> attn bf16
  proj (attn^T stationary) + residual -> out1 fp32
  LN2 -> h2^T; z^T = W1-stationary @ h2^T; relu+b1 fused in evacuation;
  out = out1 + r^T-stationary @ W2  (interleaved per 512-token chunk)
"""

import os
import sys

import numpy as np

for _p in ("/opt/trn_rl_repo", "/root/.axon_site/_ro/trn_rl_repo"):
    if os.path.isdir(_p) and _p not in sys.path:
        sys.path.insert(0, _p)

import ml_dtypes

BF16 = ml_dtypes.bfloat16

N_EMBD = 384
N_HEAD = 6
HEAD = 64
B_FULL, T = 64, 256
N_CORES = 8
B_LOC = B_FULL // N_CORES          # 8 batches per core
TOK = B_LOC * T                    # 2048 tokens per core
NT = TOK // 128                    # 16 token tiles
EPS = 1e-6
SCALE = float(N_EMBD) ** -0.5

_prog_cache: dict = {}
last_results = None


def _legalize_waits(data):
    """Split >2 semaphore waits per instruction (ISA limit) onto inserted
    EventSemaphore sync instructions immediately preceding the offender."""
    n = 0
    for fn in data["functions"]:
        for blk in fn["blocks"]:
            out = []
            for inst in blk["instructions"]:
                si = inst.get("sync_info")
                waits = (si or {}).get("on_wait") or []
                if len(waits) > 1 and inst.get("opcode") != "EventSemaphore":
                    # reg-mode waits must stay on the instruction; the HW
                    # instruction encodes a single wait, so move the rest
                    # onto preceding EventSemaphore sync ops (1 wait each)
                    reg = [w for w in waits if "imm" not in str(w.get("wait_mode"))]
                    imm = [w for w in waits if "imm" in str(w.get("wait_mode"))]
                    if reg:
                        keep, move = reg[:1], reg[1:] + imm
                    else:
                        keep, move = imm[-1:], imm[:-1]
                    for w in move:
                        out.append({
                            "debug": inst.get("debug", 0),
                            "engine": inst["engine"],
                            "ins": [],
                            "name": f"waitsplit_{n}",
                            "opcode": "EventSemaphore",
                            "outs": [],
                            "sync_info": {
                                "on_update": [],
                                "on_wait": [w],
                            },
                        })
                        n += 1
                    si["on_wait"] = keep
                out.append(inst)
            blk["instructions"] = out
    return data


_patched_tjb = False


def _patch_to_json_bytes(bass):
    global _patched_tjb
    if _patched_tjb:
        return
    _patched_tjb = True
    import orjson

    orig = bass.Bass.to_json_bytes

    def patched(self):
        return orjson.dumps(_legalize_waits(orjson.loads(orig(self))))

    bass.Bass.to_json_bytes = patched


def _build_program(has_qkb, has_vb, has_bproj, has_b2):
    import concourse.bass as bass
    import concourse.mybir as mybir
    import concourse.tile as tile
    from concourse.masks import make_identity, make_upper_triangular

    _patch_to_json_bytes(bass)

    f32 = mybir.dt.float32
    bf16 = mybir.dt.bfloat16
    AF = mybir.ActivationFunctionType
    OP = mybir.AluOpType

    nc = bass.Bass(target_bir_lowering=False, trn_type="TRN2")

    x_h = nc.dram_tensor("x", [NT, 128, N_EMBD], f32, kind="ExternalInput")
    wqk_h = nc.dram_tensor("wqk", [3, 128, 768], bf16, kind="ExternalInput")
    wv_h = nc.dram_tensor("wv", [3, 128, 384], bf16, kind="ExternalInput")
    wproj_h = nc.dram_tensor("wproj", [3, 128, 384], bf16, kind="ExternalInput")
    w1_h = nc.dram_tensor("w1", [3, 128, 1536], bf16, kind="ExternalInput")
    w2_h = nc.dram_tensor("w2", [12, 128, 384], bf16, kind="ExternalInput")
    b1_h = nc.dram_tensor("b1", [128, 12], f32, kind="ExternalInput")
    if has_qkb:
        qkb_h = nc.dram_tensor("qkb", [128, 6], f32, kind="ExternalInput")
    if has_vb:
        vb_h = nc.dram_tensor("vb", [1, 384], f32, kind="ExternalInput")
    if has_bproj:
        bpr_h = nc.dram_tensor("bpr", [1, 384], bf16, kind="ExternalInput")
    if has_b2:
        b2r_h = nc.dram_tensor("b2r", [1, 384], bf16, kind="ExternalInput")
    out_h = nc.dram_tensor("out", [NT, 128, N_EMBD], f32, kind="ExternalOutput")

    with tile.TileContext(nc) as tc:
        from contextlib import ExitStack

        es = ExitStack()
        with es:
            consts = es.enter_context(tc.tile_pool(name="consts", bufs=1))
            persist = es.enter_context(tc.tile_pool(name="persist", bufs=1))
            small = es.enter_context(tc.tile_pool(name="small", bufs=16))
            work = es.enter_context(tc.tile_pool(name="work", bufs=4))

            # ---- constants & weights (resident) ----
            ident = consts.tile([128, 128], bf16, tag="ident")
            make_identity(nc, ident)
            mask = consts.tile([128, 128], bf16, tag="mask")
            make_upper_triangular(nc, mask, val=1.0, diag=True)
            eps_t = consts.tile([128, 1], f32, tag="eps")
            nc.vector.memset(eps_t, EPS)
            # warm up the ACT table set (natural_log_exp_and_others) with a
            # minimal-dependency op so the inserted table load compiles
            warm = consts.tile([128, 1], f32, tag="warm")
            nc.scalar.activation(warm, eps_t, AF.Ln, scale=1.0)

            wqk_sb = consts.tile([128, 3, 768], bf16, tag="wqk")
            wv_sb = consts.tile([128, 3, 384], bf16, tag="wv")
            wproj_sb = consts.tile([128, 3, 384], bf16, tag="wproj")
            w1_sb = consts.tile([128, 3, 1536], bf16, tag="w1")
            w2_sb = consts.tile([128, 12, 384], bf16, tag="w2")
            b1_sb = consts.tile([128, 12], f32, tag="b1")
            for k in range(3):
                nc.sync.dma_start(wqk_sb[:, k, :], wqk_h[k])
                nc.sync.dma_start(wv_sb[:, k, :], wv_h[k])
                nc.sync.dma_start(wproj_sb[:, k, :], wproj_h[k])
                nc.sync.dma_start(w1_sb[:, k, :], w1_h[k])
            for k in range(12):
                nc.sync.dma_start(w2_sb[:, k, :], w2_h[k])
            nc.sync.dma_start(b1_sb, b1_h[:, :])
            if has_qkb:
                qkb_sb = consts.tile([128, 6], f32, tag="qkb")
                nc.sync.dma_start(qkb_sb, qkb_h[:, :])
            if has_vb:
                vb_sb = consts.tile([128, 384], f32, tag="vb")
                nc.sync.dma_start(
                    vb_sb, vb_h[:, :].to_broadcast((128, 384))
                )
            if has_bproj or has_b2:
                ones_row = consts.tile([1, 128], bf16, tag="ones_row")
                nc.vector.memset(ones_row, 1.0)
            if has_bproj:
                bpr_sb = consts.tile([1, 384], bf16, tag="bpr")
                nc.sync.dma_start(bpr_sb, bpr_h[:, :])
            if has_b2:
                b2r_sb = consts.tile([1, 384], bf16, tag="b2r")
                nc.sync.dma_start(b2r_sb, b2r_h[:, :])

            # ---- persistent activations ----
            x_all = persist.tile([128, NT, 384], f32, tag="x_all")
            hT_all = persist.tile([128, 3, TOK], bf16, tag="hT_all")
            qkT_all = persist.tile([128, 6, TOK], bf16, tag="qkT_all")
            vaug_all = persist.tile([128, NT, 6, 65], bf16, tag="vaug_all")
            attn_all = persist.tile([128, NT, 384], bf16, tag="attn_all")
            out1_all = persist.tile([128, NT, 384], f32, tag="out1_all")
            h2T_all = persist.tile([128, 3, TOK], bf16, tag="h2T_all")

            def layer_norm(src_ap, dst_bf16):
                """token-major LN: dst = (src - mean) * rsqrt(var + eps)"""
                stats = small.tile([128, 6], f32, tag="stats")
                nc.vector.bn_stats(stats, src_ap)
                mv = small.tile([128, 2], f32, tag="mv")
                nc.vector.bn_aggr(mv, stats)
                # rsqrt(v + eps) = exp(-0.5 * ln(v + eps)); keeps every ACT
                # call in one table set (no mid-kernel table reloads)
                lnv = small.tile([128, 1], f32, tag="lnv")
                nc.scalar.activation(lnv, mv[:, 1:2], AF.Ln, bias=eps_t, scale=1.0)
                rs = small.tile([128, 1], f32, tag="rs")
                nc.scalar.activation(rs, lnv, AF.Exp, scale=-0.5)
                nc.vector.tensor_scalar(
                    dst_bf16, src_ap, mv[:, 0:1], rs, OP.subtract, OP.mult
                )

            # ========== P1: load x, LN1, transpose, V (per tile) ==========
            # ones for the softmax denominator column
            nc.vector.memset(vaug_all[:, :, :, 64:65], 1.0)
            with (
                tc.tile_pool(name="ps_tr1", bufs=2, space="PSUM") as ptr,
                tc.tile_pool(name="ps_qv", bufs=3, space="PSUM") as pqv,
            ):
                # dummy transposes during the initial DMA/LN window: keeps the
                # PE HAM clock-gate busy so real matmuls start at 2.4 GHz
                for wd in range(16):
                    wtr = ptr.tile([128, 3, 128], bf16, tag="tr",
                                   name=f"warmtr_{wd}")
                    nc.tensor.transpose(wtr[:, 0, :], ident, ident)
                for i in range(NT):
                    nc.gpsimd.dma_start(x_all[:, i, :], x_h[i])
                    h_t = work.tile([128, 384], bf16, tag="h_t")
                    layer_norm(x_all[:, i, :], h_t)
                    tr = ptr.tile([128, 3, 128], bf16, tag="tr")
                    for c in range(3):
                        nc.tensor.transpose(
                            tr[:, c, :], h_t[:, c * 128:(c + 1) * 128], ident
                        )
                    nc.vector.tensor_copy(hT_all[:, :, i * 128:(i + 1) * 128], tr)
                    # V for this tile: V[s, d] = h-tile-stationary @ Wv
                    pv = pqv.tile([128, 384], f32, tag="v")
                    for kc in range(3):
                        nc.tensor.matmul(
                            pv,
                            hT_all[:, kc, i * 128:(i + 1) * 128],
                            wv_sb[:, kc, :],
                            start=(kc == 0),
                            stop=(kc == 2),
                        )
                    pv3 = pv.rearrange("p (h d) -> p h d", h=6)
                    if has_vb:
                        vtmp = work.tile([128, 384], f32, tag="vtmp")
                        nc.vector.tensor_tensor(vtmp, pv, vb_sb, OP.add)
                        nc.any.tensor_copy(
                            vaug_all[:, i, :, 0:64],
                            vtmp.rearrange("p (h d) -> p h d", h=6),
                        )
                    else:
                        nc.scalar.copy(vaug_all[:, i, :, 0:64], pv3)
                    # QK for a finished 512-token chunk (4 tiles)
                    if i % 4 == 3:
                        tch = i // 4
                        sl = slice(tch * 512, (tch + 1) * 512)
                        for m in range(6):
                            pq = pqv.tile([128, 512], f32, tag="q")
                            for kc in range(3):
                                nc.tensor.matmul(
                                    pq,
                                    wqk_sb[:, kc, m * 128:(m + 1) * 128],
                                    hT_all[:, kc, sl],
                                    start=(kc == 0),
                                    stop=(kc == 2),
                                )
                            if has_qkb:
                                nc.vector.tensor_scalar(
                                    qkT_all[:, m, sl], pq, qkb_sb[:, m:m + 1],
                                    None, OP.add,
                                )
                            else:
                                nc.scalar.copy(qkT_all[:, m, sl], pq)

            # ================= P4: attention per batch =================
            with tc.tile_pool(name="ps_att", bufs=1, space="PSUM") as patt:
                for b in range(B_LOC):
                    av = [
                        patt.tile(
                            [128, 6, 65], f32, tag=f"av{tt}", bufs=1,
                            name=f"av{tt}_{b}",
                        )
                        for tt in range(2)
                    ]
                    for r in range(2):
                        pw = patt.tile([128, 3, 512], f32, tag="w", bufs=2)
                        for hr in range(3):
                            h = r * 3 + hr
                            koff = 64 * (h % 2)
                            km, qm = 3 + h // 2, h // 2
                            t0 = b * 256
                            lhs_k0 = qkT_all[koff:koff + 64, km, t0:t0 + 128]
                            lhs_k1 = qkT_all[koff:koff + 64, km, t0 + 128:t0 + 256]
                            rhs_q = qkT_all[koff:koff + 64, qm, t0:t0 + 256]
                            rhs_qh = qkT_all[koff:koff + 64, qm, t0 + 128:t0 + 256]
                            nc.tensor.matmul(
                                pw[:, hr, 0:256], lhs_k0, rhs_q, start=True, stop=True
                            )
                            nc.tensor.matmul(
                                pw[:, hr, 256:384], lhs_k1, rhs_qh,
                                start=True, stop=True,
                            )
                        E = work.tile([128, 3, 384], bf16, tag="E", bufs=4)
                        nc.scalar.activation(E, pw[:, :, 0:384], AF.Exp, scale=SCALE)
                        # causal mask on the two diagonal blocks (cols 0:128
                        # and 256:384) in one strided multiply
                        ev = E.rearrange("p h (k c) -> p h k c", c=128)[:, :, 0::2, :]
                        mb = mask[:, None, None, :].to_broadcast((128, 3, 2, 128))
                        nc.vector.tensor_tensor(ev, ev, mb, OP.mult)
                        for hr in range(3):
                            h = r * 3 + hr
                            nc.tensor.matmul(
                                av[0][:, h, :],
                                E[:, hr, 0:128],
                                vaug_all[:, 2 * b, h, :],
                                start=True,
                                stop=True,
                            )
                            nc.tensor.matmul(
                                av[1][:, h, :],
                                E[:, hr, 128:256],
                                vaug_all[:, 2 * b, h, :],
                                start=True,
                                stop=False,
                            )
                            nc.tensor.matmul(
                                av[1][:, h, :],
                                E[:, hr, 256:384],
                                vaug_all[:, 2 * b + 1, h, :],
                                start=False,
                                stop=True,
                            )
                    for tt in range(2):
                        i = 2 * b + tt
                        rc = small.tile([128, 6], f32, tag="rc")
                        nc.vector.reciprocal(rc, av[tt][:, :, 64])
                        nc.vector.tensor_tensor(
                            attn_all[:, i, :].rearrange("p (h d) -> p h d", h=6),
                            av[tt][:, :, 0:64],
                            rc[:, :, None].to_broadcast((128, 6, 64)),
                            OP.mult,
                        )

            # ========= P4b+P5: proj + residual + LN2 + transpose =========
            with tc.tile_pool(name="ps_proj", bufs=1, space="PSUM") as ppr:
                for i in range(NT):
                    tr = ppr.tile([128, 3, 128], bf16, tag="tr2", bufs=3)
                    for c in range(3):
                        nc.tensor.transpose(
                            tr[:, c, :], attn_all[:, i, c * 128:(c + 1) * 128], ident
                        )
                    aT = work.tile([128, 3, 128], bf16, tag="aT", bufs=4)
                    nc.scalar.copy(aT, tr)
                    py = ppr.tile([128, 384], f32, tag="y", bufs=3)
                    for c in range(3):
                        nc.tensor.matmul(
                            py, aT[:, c, :], wproj_sb[:, c, :],
                            start=(c == 0), stop=(c == 2 and not has_bproj),
                        )
                    if has_bproj:
                        nc.tensor.matmul(
                            py, ones_row, bpr_sb, start=False, stop=True
                        )
                    nc.any.tensor_tensor(
                        out1_all[:, i, :], x_all[:, i, :], py, OP.add
                    )
                    h2_t = work.tile([128, 384], bf16, tag="h2_t")
                    layer_norm(out1_all[:, i, :], h2_t)
                    tr5 = ppr.tile([128, 3, 128], bf16, tag="tr5", bufs=2)
                    for c in range(3):
                        nc.tensor.transpose(
                            tr5[:, c, :], h2_t[:, c * 128:(c + 1) * 128], ident
                        )
                    nc.scalar.copy(h2T_all[:, :, i * 128:(i + 1) * 128], tr5)

            # ================= P6+P7: FFN, residual, store =================
            with (
                tc.tile_pool(name="ps_ffn", bufs=1, space="PSUM") as pffn,
                tc.tile_pool(name="rT_pool", bufs=2) as rp,
            ):
                for tch in range(4):
                    rT = rp.tile([128, 12, 512], bf16, tag="rT")
                    sl = slice(tch * 512, (tch + 1) * 512)
                    for m in range(12):
                        pz = pffn.tile([128, 512], f32, tag="z", bufs=4)
                        for kc in range(3):
                            nc.tensor.matmul(
                                pz,
                                w1_sb[:, kc, m * 128:(m + 1) * 128],
                                h2T_all[:, kc, sl],
                                start=(kc == 0),
                                stop=(kc == 2),
                            )
                        if m % 4 != 3:
                            nc.scalar.activation(
                                rT[:, m, :], pz, AF.Relu,
                                bias=b1_sb[:, m:m + 1], scale=1.0,
                            )
                        else:
                            nc.vector.tensor_scalar(
                                rT[:, m, :], pz, b1_sb[:, m:m + 1], 0.0,
                                OP.add, OP.max,
                            )
                    for il in range(4):
                        i = tch * 4 + il
                        po = pffn.tile([128, 384], f32, tag="o", bufs=3)
                        for kc in range(12):
                            nc.tensor.matmul(
                                po,
                                rT[:, kc, il * 128:(il + 1) * 128],
                                w2_sb[:, kc, :],
                                start=(kc == 0),
                                stop=(kc == 11 and not has_b2),
                            )
                        if has_b2:
                            nc.tensor.matmul(
                                po, ones_row, b2r_sb, start=False, stop=True
                            )
                        o_t = work.tile([128, 384], f32, tag="o_t")
                        nc.any.tensor_tensor(o_t, out1_all[:, i, :], po, OP.add)
                        nc.gpsimd.dma_start(out_h[i], o_t)

    return nc


def kernel(x, ln1_scale, ln1_bias, Wq, Wk, Wv, Wproj, bproj,
           ln2_scale, ln2_bias, W1, b1, W2, b2):
    global last_results
    x = np.asarray(x, np.float32)
    g1 = np.asarray(ln1_scale, np.float32)
    be1 = np.asarray(ln1_bias, np.float32)
    Wq = np.asarray(Wq, np.float32)
    Wk = np.asarray(Wk, np.float32)
    Wv = np.asarray(Wv, np.float32)
    Wproj = np.asarray(Wproj, np.float32)
    bproj = np.asarray(bproj, np.float32)
    g2 = np.asarray(ln2_scale, np.float32)
    be2 = np.asarray(ln2_bias, np.float32)
    W1 = np.asarray(W1, np.float32)
    b1 = np.asarray(b1, np.float32)
    W2 = np.asarray(W2, np.float32)
    b2 = np.asarray(b2, np.float32)

    # [H, C, D] -> [C, H*D]
    wq_f = Wq.transpose(1, 0, 2).reshape(N_EMBD, N_EMBD)
    wk_f = Wk.transpose(1, 0, 2).reshape(N_EMBD, N_EMBD)
    wv_f = Wv.transpose(1, 0, 2).reshape(N_EMBD, N_EMBD)

    # fold LN scales into the weights that consume the normalized stream
    wqk = np.concatenate([g1[:, None] * wq_f, g1[:, None] * wk_f], axis=1)
    wv_s = g1[:, None] * wv_f
    w1_s = g2[:, None] * W1

    qk_bias = be1 @ np.concatenate([wq_f, wk_f], axis=1)      # [768]
    v_bias = be1 @ wv_f                                       # [384]
    b1_eff = b1 + be2 @ W1                                    # [1536]

    has_qkb = bool(np.any(qk_bias != 0.0))
    has_vb = bool(np.any(v_bias != 0.0))
    has_bproj = bool(np.any(bproj != 0.0))
    has_b2 = bool(np.any(b2 != 0.0))

    key = (has_qkb, has_vb, has_bproj, has_b2)
    if key not in _prog_cache:
        _prog_cache[key] = _build_program(*key)
    nc = _prog_cache[key]

    base = {
        "wqk": np.ascontiguousarray(wqk.reshape(3, 128, 768)).astype(BF16),
        "wv": np.ascontiguousarray(wv_s.reshape(3, 128, 384)).astype(BF16),
        "wproj": np.ascontiguousarray(Wproj.reshape(3, 128, 384)).astype(BF16),
        "w1": np.ascontiguousarray(w1_s.reshape(3, 128, 1536)).astype(BF16),
        "w2": np.ascontiguousarray(W2.reshape(12, 128, 384)).astype(BF16),
        "b1": np.ascontiguousarray(b1_eff.reshape(12, 128).T).astype(np.float32),
    }
    if has_qkb:
        base["qkb"] = np.ascontiguousarray(qk_bias.reshape(6, 128).T).astype(
            np.float32
        )
    if has_vb:
        base["vb"] = v_bias.reshape(1, 384).astype(np.float32)
    if has_bproj:
        base["bpr"] = bproj.reshape(1, 384).astype(BF16)
    if has_b2:
        base["b2r"] = b2.reshape(1, 384).astype(BF16)

    x_r = x.reshape(N_CORES, NT, 128, N_EMBD)
    in_maps = [dict(base, x=np.ascontiguousarray(x_r[c])) for c in range(N_CORES)]

    from concourse.bass_utils import run_bass_kernel_spmd

    last_results = run_bass_kernel_spmd(
        nc, in_maps, core_ids=list(range(N_CORES))
    )
    out = np.concatenate(
        [r["out"].reshape(B_LOC, T, N_EMBD) for r in last_results.results], axis=0
    )
    return out


# revision 69
# speedup vs baseline: 1.0077x; 1.0077x over previous
"""Trainium2 Bass kernel for a pre-LN transformer block (attention + FFN).

Full inputs in, full outputs out. Data-parallel over the batch dim across
8 NeuronCores (8 batches/core), weights replicated, no collectives.

Per-core dataflow (2048 tokens = 16 tiles of 128):
  LN1 (token-major, bn_stats) -> h bf16 -> PE-transpose -> h^T [c, t]
  Q^T,K^T = Wqk^T @ h^T (feature-major [d, t]); V = h^T-stationary @ Wv
  (token-major [s, d], ones column appended for softmax denominators)
  wei^T[s,t] = K^T-slices stationary @ Q^T; E = exp(wei * C^-0.5) via ACT
  (logits are O(0.3) so no max-subtraction needed); causal mask = one
  upper-triangular multiply per diagonal block
  attn[t, 64+1] = E-stationary @ [V | 1]  (numerator + denominator in one
  accumulation, token-major) -> reciprocal + scale -> attn bf16
  proj (attn^T stationary) + residual -> out1 fp32
  LN2 -> h2^T; z^T = W1-stationary @ h2^T; relu+b1 fused in evacuation;
  out = out1 + r^T-stationary @ W2  (interleaved per 512-token chunk)
"""

import os
import sys

import numpy as np

for _p in ("/opt/trn_rl_repo", "/root/.axon_site/_ro/trn_rl_repo"):
    if os.path.isdir(_p) and _p not in sys.path:
        sys.path.insert(0, _p)

import ml_dtypes

BF16 = ml_dtypes.bfloat16

N_EMBD = 384
N_HEAD = 6
HEAD = 64
B_FULL, T = 64, 256
N_CORES = 8
B_LOC = B_FULL // N_CORES          # 8 batches per core
TOK = B_LOC * T                    # 2048 tokens per core
NT = TOK // 128                    # 16 token tiles
EPS = 1e-6
SCALE = float(N_EMBD) ** -0.5

_prog_cache: dict = {}
last_results = None


def _legalize_waits(data):
    """Split >2 semaphore waits per instruction (ISA limit) onto inserted
    EventSemaphore sync instructions immediately preceding the offender."""
    n = 0
    for fn in data["functions"]:
        for blk in fn["blocks"]:
            out = []
            for inst in blk["instructions"]:
                si = inst.get("sync_info")
                waits = (si or {}).get("on_wait") or []
                if len(waits) > 1 and inst.get("opcode") != "EventSemaphore":
                    # reg-mode waits must stay on the instruction; the HW
                    # instruction encodes a single wait, so move the rest
                    # onto preceding EventSemaphore sync ops (1 wait each)
                    reg = [w for w in waits if "imm" not in str(w.get("wait_mode"))]
                    imm = [w for w in waits if "imm" in str(w.get("wait_mode"))]
                    if reg:
                        keep, move = reg[:1], reg[1:] + imm
                    else:
                        keep, move = imm[-1:], imm[:-1]
                    for w in move:
                        out.append({
                            "debug": inst.get("debug", 0),
                            "engine": inst["engine"],
                            "ins": [],
                            "name": f"waitsplit_{n}",
                            "opcode": "EventSemaphore",
                            "outs": [],
                            "sync_info": {
                                "on_update": [],
                                "on_wait": [w],
                            },
                        })
                        n += 1
                    si["on_wait"] = keep
                out.append(inst)
            blk["instructions"] = out
    return data


_patched_tjb = False


def _patch_to_json_bytes(bass):
    global _patched_tjb
    if _patched_tjb:
        return
    _patched_tjb = True
    import orjson

    orig = bass.Bass.to_json_bytes

    def patched(self):
        return orjson.dumps(_legalize_waits(orjson.loads(orig(self))))

    bass.Bass.to_json_bytes = patched


def _build_program(has_qkb, has_vb, has_bproj, has_b2):
    import concourse.bass as bass
    import concourse.mybir as mybir
    import concourse.tile as tile
    from concourse.masks import make_identity, make_upper_triangular

    _patch_to_json_bytes(bass)

    f32 = mybir.dt.float32
    bf16 = mybir.dt.bfloat16
    AF = mybir.ActivationFunctionType
    OP = mybir.AluOpType

    nc = bass.Bass(target_bir_lowering=False, trn_type="TRN2")

    x_h = nc.dram_tensor("x", [NT, 128, N_EMBD], f32, kind="ExternalInput")
    wqk_h = nc.dram_tensor("wqk", [3, 128, 768], bf16, kind="ExternalInput")
    wv_h = nc.dram_tensor("wv", [3, 128, 384], bf16, kind="ExternalInput")
    wproj_h = nc.dram_tensor("wproj", [3, 128, 384], bf16, kind="ExternalInput")
    w1_h = nc.dram_tensor("w1", [3, 128, 1536], bf16, kind="ExternalInput")
    w2_h = nc.dram_tensor("w2", [12, 128, 384], bf16, kind="ExternalInput")
    b1_h = nc.dram_tensor("b1", [128, 12], f32, kind="ExternalInput")
    if has_qkb:
        qkb_h = nc.dram_tensor("qkb", [128, 6], f32, kind="ExternalInput")
    if has_vb:
        vb_h = nc.dram_tensor("vb", [1, 384], f32, kind="ExternalInput")
    if has_bproj:
        bpr_h = nc.dram_tensor("bpr", [1, 384], bf16, kind="ExternalInput")
    if has_b2:
        b2r_h = nc.dram_tensor("b2r", [1, 384], bf16, kind="ExternalInput")
    out_h = nc.dram_tensor("out", [NT, 128, N_EMBD], f32, kind="ExternalOutput")

    with tile.TileContext(nc) as tc:
        from contextlib import ExitStack

        es = ExitStack()
        with es:
            consts = es.enter_context(tc.tile_pool(name="consts", bufs=1))
            persist = es.enter_context(tc.tile_pool(name="persist", bufs=1))
            small = es.enter_context(tc.tile_pool(name="small", bufs=16))
            work = es.enter_context(tc.tile_pool(name="work", bufs=4))

            # ---- constants & weights (resident) ----
            ident = consts.tile([128, 128], bf16, tag="ident")
            make_identity(nc, ident)
            mask = consts.tile([128, 128], bf16, tag="mask")
            make_upper_triangular(nc, mask, val=1.0, diag=True)
            eps_t = consts.tile([128, 1], f32, tag="eps")
            nc.vector.memset(eps_t, EPS)
            # warm up the ACT table set (natural_log_exp_and_others) with a
            # minimal-dependency op so the inserted table load compiles
            warm = consts.tile([128, 1], f32, tag="warm")
            nc.scalar.activation(warm, eps_t, AF.Ln, scale=1.0)

            wqk_sb = consts.tile([128, 3, 768], bf16, tag="wqk")
            wv_sb = consts.tile([128, 3, 384], bf16, tag="wv")
            wproj_sb = consts.tile([128, 3, 384], bf16, tag="wproj")
            w1_sb = consts.tile([128, 3, 1536], bf16, tag="w1")
            w2_sb = consts.tile([128, 12, 384], bf16, tag="w2")
            b1_sb = consts.tile([128, 12], f32, tag="b1")
            for k in range(3):
                nc.sync.dma_start(wqk_sb[:, k, :], wqk_h[k])
                nc.sync.dma_start(wv_sb[:, k, :], wv_h[k])
                nc.sync.dma_start(wproj_sb[:, k, :], wproj_h[k])
                nc.sync.dma_start(w1_sb[:, k, :], w1_h[k])
            for k in range(12):
                nc.sync.dma_start(w2_sb[:, k, :], w2_h[k])
            nc.sync.dma_start(b1_sb, b1_h[:, :])
            if has_qkb:
                qkb_sb = consts.tile([128, 6], f32, tag="qkb")
                nc.sync.dma_start(qkb_sb, qkb_h[:, :])
            if has_vb:
                vb_sb = consts.tile([128, 384], f32, tag="vb")
                nc.sync.dma_start(
                    vb_sb, vb_h[:, :].to_broadcast((128, 384))
                )
            if has_bproj or has_b2:
                ones_row = consts.tile([1, 128], bf16, tag="ones_row")
                nc.vector.memset(ones_row, 1.0)
            if has_bproj:
                bpr_sb = consts.tile([1, 384], bf16, tag="bpr")
                nc.sync.dma_start(bpr_sb, bpr_h[:, :])
            if has_b2:
                b2r_sb = consts.tile([1, 384], bf16, tag="b2r")
                nc.sync.dma_start(b2r_sb, b2r_h[:, :])

            # ---- persistent activations ----
            x_all = persist.tile([128, NT, 384], f32, tag="x_all")
            hT_all = persist.tile([128, 3, TOK], bf16, tag="hT_all")
            qkT_all = persist.tile([128, 6, TOK], bf16, tag="qkT_all")
            vaug_all = persist.tile([128, NT, 6, 65], bf16, tag="vaug_all")
            attn_all = persist.tile([128, NT, 384], bf16, tag="attn_all")
            out1_all = persist.tile([128, NT, 384], f32, tag="out1_all")
            h2T_all = persist.tile([128, 3, TOK], bf16, tag="h2T_all")

            def layer_norm(src_ap, dst_bf16):
                """token-major LN: dst = (src - mean) * rsqrt(var + eps)"""
                stats = small.tile([128, 6], f32, tag="stats")
                nc.vector.bn_stats(stats, src_ap)
                mv = small.tile([128, 2], f32, tag="mv")
                nc.vector.bn_aggr(mv, stats)
                # rsqrt(v + eps) = exp(-0.5 * ln(v + eps)); keeps every ACT
                # call in one table set (no mid-kernel table reloads)
                lnv = small.tile([128, 1], f32, tag="lnv")
                nc.scalar.activation(lnv, mv[:, 1:2], AF.Ln, bias=eps_t, scale=1.0)
                rs = small.tile([128, 1], f32, tag="rs")
                nc.scalar.activation(rs, lnv, AF.Exp, scale=-0.5)
                nc.gpsimd.tensor_scalar(
                    dst_bf16, src_ap, mv[:, 0:1], rs, OP.subtract, OP.mult
                )

            # ========== P1: load x, LN1, transpose, V (per tile) ==========
            # ones for the softmax denominator column
            nc.vector.memset(vaug_all[:, :, :, 64:65], 1.0)
            with (
                tc.tile_pool(name="ps_tr1", bufs=2, space="PSUM") as ptr,
                tc.tile_pool(name="ps_qv", bufs=4, space="PSUM") as pqv,
            ):
                # dummy transposes during the initial DMA/LN window: keeps the
                # PE HAM clock-gate busy so real matmuls start at 2.4 GHz
                for wd in range(16):
                    wtr = ptr.tile([128, 3, 128], bf16, tag="tr",
                                   name=f"warmtr_{wd}")
                    nc.tensor.transpose(wtr[:, 0, :], ident, ident)
                for i in range(NT):
                    (nc.scalar if i % 2 == 0 else nc.gpsimd).dma_start(
                        x_all[:, i, :], x_h[i]
                    )
                    h_t = work.tile([128, 384], bf16, tag="h_t")
                    layer_norm(x_all[:, i, :], h_t)
                    tr = ptr.tile([128, 3, 128], bf16, tag="tr")
                    for c in range(3):
                        nc.tensor.transpose(
                            tr[:, c, :], h_t[:, c * 128:(c + 1) * 128], ident
                        )
                    nc.vector.tensor_copy(hT_all[:, :, i * 128:(i + 1) * 128], tr)
                    # V for this tile: V[s, d] = h-tile-stationary @ Wv
                    pv = pqv.tile([128, 384], f32, tag="v", bufs=2)
                    for kc in range(3):
                        nc.tensor.matmul(
                            pv,
                            hT_all[:, kc, i * 128:(i + 1) * 128],
                            wv_sb[:, kc, :],
                            start=(kc == 0),
                            stop=(kc == 2),
                        )
                    pv3 = pv.rearrange("p (h d) -> p h d", h=6)
                    if has_vb:
                        vtmp = work.tile([128, 384], f32, tag="vtmp")
                        nc.vector.tensor_tensor(vtmp, pv, vb_sb, OP.add)
                        nc.any.tensor_copy(
                            vaug_all[:, i, :, 0:64],
                            vtmp.rearrange("p (h d) -> p h d", h=6),
                        )
                    else:
                        nc.scalar.copy(vaug_all[:, i, :, 0:64], pv3)
                    # QK for a finished 512-token chunk (4 tiles)
                    if i % 4 == 3:
                        tch = i // 4
                        sl = slice(tch * 512, (tch + 1) * 512)
                        for m in range(6):
                            pq = pqv.tile([128, 512], f32, tag="q")
                            for kc in range(3):
                                nc.tensor.matmul(
                                    pq,
                                    wqk_sb[:, kc, m * 128:(m + 1) * 128],
                                    hT_all[:, kc, sl],
                                    start=(kc == 0),
                                    stop=(kc == 2),
                                )
                            if has_qkb:
                                nc.vector.tensor_scalar(
                                    qkT_all[:, m, sl], pq, qkb_sb[:, m:m + 1],
                                    None, OP.add,
                                )
                            else:
                                nc.scalar.copy(qkT_all[:, m, sl], pq)

            # ================= P4: attention per batch =================
            with tc.tile_pool(name="ps_att", bufs=1, space="PSUM") as patt:
                for b in range(B_LOC):
                    av = [
                        patt.tile(
                            [128, 6, 65], f32, tag=f"av{tt}", bufs=1,
                            name=f"av{tt}_{b}",
                        )
                        for tt in range(2)
                    ]
                    for r in range(2):
                        pw = patt.tile([128, 3, 512], f32, tag="w", bufs=2)
                        for hr in range(3):
                            h = r * 3 + hr
                            koff = 64 * (h % 2)
                            km, qm = 3 + h // 2, h // 2
                            t0 = b * 256
                            lhs_k0 = qkT_all[koff:koff + 64, km, t0:t0 + 128]
                            lhs_k1 = qkT_all[koff:koff + 64, km, t0 + 128:t0 + 256]
                            rhs_q = qkT_all[koff:koff + 64, qm, t0:t0 + 256]
                            rhs_qh = qkT_all[koff:koff + 64, qm, t0 + 128:t0 + 256]
                            nc.tensor.matmul(
                                pw[:, hr, 0:256], lhs_k0, rhs_q, start=True, stop=True
                            )
                            nc.tensor.matmul(
                                pw[:, hr, 256:384], lhs_k1, rhs_qh,
                                start=True, stop=True,
                            )
                        E = work.tile([128, 3, 384], bf16, tag="E", bufs=4)
                        nc.scalar.activation(E, pw[:, :, 0:384], AF.Exp, scale=SCALE)
                        # causal mask on the two diagonal blocks (cols 0:128
                        # and 256:384) in one strided multiply
                        ev = E.rearrange("p h (k c) -> p h k c", c=128)[:, :, 0::2, :]
                        mb = mask[:, None, None, :].to_broadcast((128, 3, 2, 128))
                        nc.vector.tensor_tensor(ev, ev, mb, OP.mult)
                        for hr in range(3):
                            h = r * 3 + hr
                            nc.tensor.matmul(
                                av[0][:, h, :],
                                E[:, hr, 0:128],
                                vaug_all[:, 2 * b, h, :],
                                start=True,
                                stop=True,
                            )
                            nc.tensor.matmul(
                                av[1][:, h, :],
                                E[:, hr, 128:256],
                                vaug_all[:, 2 * b, h, :],
                                start=True,
                                stop=False,
                            )
                            nc.tensor.matmul(
                                av[1][:, h, :],
                                E[:, hr, 256:384],
                                vaug_all[:, 2 * b + 1, h, :],
                                start=False,
                                stop=True,
                            )
                    for tt in range(2):
                        i = 2 * b + tt
                        rc = small.tile([128, 6], f32, tag="rc")
                        nc.vector.reciprocal(rc, av[tt][:, :, 64])
                        nc.vector.tensor_tensor(
                            attn_all[:, i, :].rearrange("p (h d) -> p h d", h=6),
                            av[tt][:, :, 0:64],
                            rc[:, :, None].to_broadcast((128, 6, 64)),
                            OP.mult,
                        )

            # ========= P4b+P5: proj + residual + LN2 + transpose =========
            with tc.tile_pool(name="ps_proj", bufs=1, space="PSUM") as ppr:
                for i in range(NT):
                    tr = ppr.tile([128, 3, 128], bf16, tag="tr2", bufs=4)
                    for c in range(3):
                        nc.tensor.transpose(
                            tr[:, c, :], attn_all[:, i, c * 128:(c + 1) * 128], ident
                        )
                    aT = work.tile([128, 3, 128], bf16, tag="aT", bufs=4)
                    nc.scalar.copy(aT, tr)
                    py = ppr.tile([128, 384], f32, tag="y", bufs=2)
                    for c in range(3):
                        nc.tensor.matmul(
                            py, aT[:, c, :], wproj_sb[:, c, :],
                            start=(c == 0), stop=(c == 2 and not has_bproj),
                        )
                    if has_bproj:
                        nc.tensor.matmul(
                            py, ones_row, bpr_sb, start=False, stop=True
                        )
                    nc.any.tensor_tensor(
                        out1_all[:, i, :], x_all[:, i, :], py, OP.add
                    )
                    h2_t = work.tile([128, 384], bf16, tag="h2_t")
                    layer_norm(out1_all[:, i, :], h2_t)
                    tr5 = ppr.tile([128, 3, 128], bf16, tag="tr5", bufs=2)
                    for c in range(3):
                        nc.tensor.transpose(
                            tr5[:, c, :], h2_t[:, c * 128:(c + 1) * 128], ident
                        )
                    nc.scalar.copy(h2T_all[:, :, i * 128:(i + 1) * 128], tr5)

            # ================= P6+P7: FFN, residual, store =================
            with (
                tc.tile_pool(name="ps_ffn", bufs=1, space="PSUM") as pffn,
                tc.tile_pool(name="rT_pool", bufs=2) as rp,
            ):
                for tch in range(4):
                    rT = rp.tile([128, 12, 512], bf16, tag="rT")
                    sl = slice(tch * 512, (tch + 1) * 512)
                    for m in range(12):
                        pz = pffn.tile([128, 512], f32, tag="z", bufs=4)
                        for kc in range(3):
                            nc.tensor.matmul(
                                pz,
                                w1_sb[:, kc, m * 128:(m + 1) * 128],
                                h2T_all[:, kc, sl],
                                start=(kc == 0),
                                stop=(kc == 2),
                            )
                        if m % 2 == 0:
                            nc.scalar.activation(
                                rT[:, m, :], pz, AF.Relu,
                                bias=b1_sb[:, m:m + 1], scale=1.0,
                            )
                        else:
                            nc.vector.tensor_scalar(
                                rT[:, m, :], pz, b1_sb[:, m:m + 1], 0.0,
                                OP.add, OP.max,
                            )
                    for il in range(4):
                        i = tch * 4 + il
                        po = pffn.tile([128, 384], f32, tag="o", bufs=3)
                        for kc in range(12):
                            nc.tensor.matmul(
                                po,
                                rT[:, kc, il * 128:(il + 1) * 128],
                                w2_sb[:, kc, :],
                                start=(kc == 0),
                                stop=(kc == 11 and not has_b2),
                            )
                        if has_b2:
                            nc.tensor.matmul(
                                po, ones_row, b2r_sb, start=False, stop=True
                            )
                        o_t = work.tile([128, 384], f32, tag="o_t")
                        nc.any.tensor_tensor(o_t, out1_all[:, i, :], po, OP.add)
                        nc.sync.dma_start(out_h[i], o_t)

    return nc


def kernel(x, ln1_scale, ln1_bias, Wq, Wk, Wv, Wproj, bproj,
           ln2_scale, ln2_bias, W1, b1, W2, b2):
    global last_results
    x = np.asarray(x, np.float32)
    g1 = np.asarray(ln1_scale, np.float32)
    be1 = np.asarray(ln1_bias, np.float32)
    Wq = np.asarray(Wq, np.float32)
    Wk = np.asarray(Wk, np.float32)
    Wv = np.asarray(Wv, np.float32)
    Wproj = np.asarray(Wproj, np.float32)
    bproj = np.asarray(bproj, np.float32)
    g2 = np.asarray(ln2_scale, np.float32)
    be2 = np.asarray(ln2_bias, np.float32)
    W1 = np.asarray(W1, np.float32)
    b1 = np.asarray(b1, np.float32)
    W2 = np.asarray(W2, np.float32)
    b2 = np.asarray(b2, np.float32)

    # [H, C, D] -> [C, H*D]
    wq_f = Wq.transpose(1, 0, 2).reshape(N_EMBD, N_EMBD)
    wk_f = Wk.transpose(1, 0, 2).reshape(N_EMBD, N_EMBD)
    wv_f = Wv.transpose(1, 0, 2).reshape(N_EMBD, N_EMBD)

    # fold LN scales into the weights that consume the normalized stream
    wqk = np.concatenate([g1[:, None] * wq_f, g1[:, None] * wk_f], axis=1)
    wv_s = g1[:, None] * wv_f
    w1_s = g2[:, None] * W1

    qk_bias = be1 @ np.concatenate([wq_f, wk_f], axis=1)      # [768]
    v_bias = be1 @ wv_f                                       # [384]
    b1_eff = b1 + be2 @ W1                                    # [1536]

    has_qkb = bool(np.any(qk_bias != 0.0))
    has_vb = bool(np.any(v_bias != 0.0))
    has_bproj = bool(np.any(bproj != 0.0))
    has_b2 = bool(np.any(b2 != 0.0))

    key = (has_qkb, has_vb, has_bproj, has_b2)
    if key not in _prog_cache:
        _prog_cache[key] = _build_program(*key)
    nc = _prog_cache[key]

    base = {
        "wqk": np.ascontiguousarray(wqk.reshape(3, 128, 768)).astype(BF16),
        "wv": np.ascontiguousarray(wv_s.reshape(3, 128, 384)).astype(BF16),
        "wproj": np.ascontiguousarray(Wproj.reshape(3, 128, 384)).astype(BF16),
        "w1": np.ascontiguousarray(w1_s.reshape(3, 128, 1536)).astype(BF16),
        "w2": np.ascontiguousarray(W2.reshape(12, 128, 384)).astype(BF16),
        "b1": np.ascontiguousarray(b1_eff.reshape(12, 128).T).astype(np.float32),
    }
    if has_qkb:
        base["qkb"] = np.ascontiguousarray(qk_bias.reshape(6, 128).T).astype(
            np.float32
        )
    if has_vb:
        base["vb"] = v_bias.reshape(1, 384).astype(np.float32)
    if has_bproj:
        base["bpr"] = bproj.reshape(1, 384).astype(BF16)
    if has_b2:
        base["b2r"] = b2.reshape(1, 384).astype(BF16)

    x_r = x.reshape(N_CORES, NT, 128, N_EMBD)
    in_maps = [dict(base, x=np.ascontiguousarray(x_r[c])) for c in range(N_CORES)]

    from concourse.bass_utils import run_bass_kernel_spmd

    last_results = run_bass_kernel_spmd(
        nc, in_maps, core_ids=list(range(N_CORES))
    )
    out = np.concatenate(
        [r["out"].reshape(B_LOC, T, N_EMBD) for r in last_results.results], axis=0
    )
    return out
